# revision 5
# baseline (speedup 1.0000x reference)
"""GAT EncodeProcessDecode (4 GAT layers) on 8 Trainium2 NeuronCores — v2.

Strategy (graph/data parallel, per sharding hint):
  - Nodes sharded contiguously across 8 cores (dst-sharding); edges sorted
    by dst on host and packed into per-dst-tile chunks of 128.
  - Per layer: phase A computes augmented rows [h | 1 | s_src | s_dst] in
    bf16 (attention scalars ride the same matmul via augmented weights, the
    per-node transpose is done by the DMA xbar), AllGather replicates the
    row table, phase B gathers h[src] rows with ONE batched indirect DMA
    per dst tile and performs the segment softmax + scatter-add as one-hot
    matmuls on the PE (PSUM accumulates [128 dst, 129]; col 128 is the
    softmax denominator driven by the constant-ones column).
  - Everything bf16 on the wire (tunnel + HBM + collective); PSUM fp32.
  - Biases are folded into the next layer's matmul (host-precomputed rank-1
    rows); the final bias is added on the host.
  - Meta is packed 4B/edge: (dst_global_row << 16) | src. Padding lanes use
    dst_row = n_pad whose s_dst slot is -1e4 so exp() underflows to exactly
    0 and they contribute nothing.
  - Host caches the compiled program AND the jitted PJRT callable keyed by
    the edge list; per-call work is just padding/casting, the tunnel
    transfer, and device execution.
"""

import sys

sys.path.insert(0, "/opt/trn_rl_repo")

import concurrent.futures as _cf

import numpy as np
from contextlib import ExitStack

import jax
from jax.experimental.shard_map import shard_map
from jax.sharding import Mesh, NamedSharding, PartitionSpec

from concourse import bass, bacc, mybir
import concourse.tile as tile
from concourse.bass2jax import (
    _bass_exec_p,
    install_neuronx_cc_hook,
    partition_id_tensor,
)

F32 = mybir.dt.float32
BF16 = mybir.dt.bfloat16
I32 = mybir.dt.int32
OP = mybir.AluOpType
ACTF = mybir.ActivationFunctionType
NPBF16 = mybir.dt.np(BF16)

P = 128
D = 128
ROWB = 144  # bf16 words per augmented row (288B, 32B aligned)
COL_ONES = 128
COL_SSRC = 129
COL_SDST = 130
NEG_SLOPE = 0.2
N_CORES = 8
N_FULL = 50000
PAD_SDST = -10000.0  # s_dst slot of padding rows; exp(lrelu(x+PAD)) == 0


def _prep_graph(edge_index, n_nodes=N_FULL, n_cores=N_CORES):
    """Sort edges (plus self loops) by dst; pack per-tile meta [P, n] int32.

    Packed word: (dst_haug_row << 16) | src_haug_row, where haug rows are
    laid out as core*(npc+128) + local_offset (each core's AllGather shard
    carries a 128-row PAD_SDST block after its npc real rows).  Padding
    lanes point at the first pad row, whose s_dst slot makes exp() == 0.
    Returns (tiles_per_core, n_pad, n_chunks, metas) with metas[c] of shape
    [P, sum(n_chunks)] int32 (columns grouped by tile).
    """
    tiles_per_core = -(-n_nodes // (n_cores * P))
    n_tiles = n_cores * tiles_per_core
    n_pad = n_tiles * P
    npc = tiles_per_core * P
    src = np.asarray(edge_index[0], dtype=np.int64)
    dst = np.asarray(edge_index[1], dtype=np.int64)
    loops = np.arange(n_nodes, dtype=np.int64)
    src = np.concatenate([src, loops])
    dst = np.concatenate([dst, loops])
    order = np.argsort(dst, kind="stable")
    src, dst = src[order], dst[order]
    counts = np.bincount(dst // P, minlength=n_tiles)
    starts = np.concatenate([[0], np.cumsum(counts)])

    n_chunks = []
    for s in range(tiles_per_core):
        m = 1
        for c in range(n_cores):
            m = max(m, -(-int(counts[c * tiles_per_core + s]) // P))
        n_chunks.append(m)

    # node id -> haug row in the (npc+128)-per-core layout
    def hrow(v):
        return (v // npc) * (npc + P) + (v % npc)

    pad_word = np.int64(npc) << 16  # core 0's first pad row
    metas = []
    for c in range(n_cores):
        cols = []
        for s in range(tiles_per_core):
            t = c * tiles_per_core + s
            n = n_chunks[s]
            e0, e1 = int(starts[t]), int(starts[t + 1])
            blk = np.full((P, n), pad_word, dtype=np.int64)
            idx = np.arange(e1 - e0)
            blk[idx % P, idx // P] = (hrow(dst[e0:e1]) << 16) | hrow(src[e0:e1])
            cols.append(blk)
        metas.append(
            np.ascontiguousarray(
                np.concatenate(cols, axis=1).astype(np.uint32).view(np.int32)
            )
        )
    return tiles_per_core, n_pad, n_chunks, metas


def _aug(w, a_s, a_d):
    w = np.asarray(w, dtype=np.float32)
    return np.ascontiguousarray(
        np.concatenate(
            [
                w,
                (w @ np.asarray(a_s, np.float32))[:, None],
                (w @ np.asarray(a_d, np.float32))[:, None],
            ],
            axis=1,
        )
    )


def _build_program(tiles_per_core, n_chunks, n_cores, debug_dump=False):
    npc = tiles_per_core * P
    npc_x = npc + P  # per-core AllGather shard incl. the 128-row pad block
    n_pad_x = n_cores * npc_x
    sum_n = sum(n_chunks)

    nc = bacc.Bacc("TRN2", target_bir_lowering=False, debug=False, num_devices=n_cores)
    dbg_haug = dbg_u0 = dbg = None
    if debug_dump:
        n0 = n_chunks[0]
        dbg_haug = nc.dram_tensor("dbg_haug", [n_pad_x, ROWB], BF16, kind="ExternalOutput").ap()
        dbg_u0 = nc.dram_tensor("dbg_u0", [npc, D], BF16, kind="ExternalOutput").ap()
        dbg = {
            "g": nc.dram_tensor("dbg_g", [P, n0 * ROWB], BF16, kind="ExternalOutput").ap(),
            "zt": nc.dram_tensor("dbg_zt", [P, n0], BF16, kind="ExternalOutput").ap(),
            "ex": nc.dram_tensor("dbg_ex", [P, n0], F32, kind="ExternalOutput").ap(),
            "srcs": nc.dram_tensor("dbg_srcs", [P, n0], I32, kind="ExternalOutput").ap(),
            "grow": nc.dram_tensor("dbg_grow", [P, n0], I32, kind="ExternalOutput").ap(),
            "z": nc.dram_tensor("dbg_z", [P, n0], F32, kind="ExternalOutput").ap(),
        }

    I8 = mybir.dt.int8
    # packed inputs: one array for x (int8 rows + f32 scale bytes), one for
    # all weights (5 aug matrices + 3 bias rows), to minimize per-array
    # tunnel round-trips
    xp_in = nc.dram_tensor("x_p", [npc, D + 4], I8, kind="ExternalInput").ap()
    meta_in = nc.dram_tensor("meta", [P, sum_n], I32, kind="ExternalInput").ap()
    wp_in = nc.dram_tensor("w_p", [5 * P + 3, D + 2], BF16, kind="ExternalInput").ap()
    y_out = nc.dram_tensor("y_p", [npc, D + 4], I8, kind="ExternalOutput").ap()

    with ExitStack() as st:
        tc = st.enter_context(tile.TileContext(nc))
        cpool = st.enter_context(tc.tile_pool(name="consts", bufs=1))
        apool = st.enter_context(tc.tile_pool(name="pha", bufs=4))
        gpool = st.enter_context(tc.tile_pool(name="gat", bufs=3))
        swpool = st.enter_context(tc.tile_pool(name="sw", bufs=8))
        epool = st.enter_context(tc.tile_pool(name="epi", bufs=8))
        pp = st.enter_context(tc.tile_pool(name="ps", bufs=2, space="PSUM"))
        ppb = st.enter_context(tc.tile_pool(name="psb", bufs=2, space="PSUM"))
        dpool = st.enter_context(tc.tile_pool(name="dramp", bufs=1, space="DRAM"))

        ag_in = dpool.tile([npc_x, ROWB], BF16, name="ag_in")
        haugs = [
            dpool.tile([n_pad_x, ROWB], BF16, addr_space="Shared", name=f"haug{i}")
            for i in range(4)
        ]
        y_mid = [dpool.tile([npc, D], BF16, name=f"ymid{i}") for i in range(3)]

        # constants
        iota_i = cpool.tile([P, P], I32, name="iota_i")
        nc.gpsimd.iota(iota_i[:], pattern=[[1, P]], base=0, channel_multiplier=0)
        iota_f = cpool.tile([P, P], F32, name="iota_f")
        nc.vector.tensor_copy(iota_f[:], iota_i[:])
        iota_p = cpool.tile([P, 1], I32, name="iota_p")
        nc.gpsimd.iota(iota_p[:], pattern=[[0, 1]], base=0, channel_multiplier=1)
        iota_pf = cpool.tile([P, 1], F32, name="iota_pf")
        nc.vector.tensor_copy(iota_pf[:], iota_p[:])
        ident_t = cpool.tile([P, P], BF16, name="ident_t")
        nc.vector.tensor_scalar(
            ident_t[:], iota_f[:], iota_pf[:, 0:1], None, op0=OP.is_equal
        )
        ones_row = cpool.tile([1, P], BF16, name="ones_row")
        nc.vector.memset(ones_row[:], 1.0)
        padrow = cpool.tile([P, ROWB], BF16, name="padrow")
        nc.vector.memset(padrow[:], PAD_SDST)
        nc.sync.dma_start(ag_in[npc:npc_x, :], padrow[:])
        w_t = []
        for i in range(5):
            wt = cpool.tile([D, D + 2], BF16, name=f"w_t{i}")
            nc.sync.dma_start(wt[:], wp_in[i * P : (i + 1) * P, :])
            w_t.append(wt)
        r_t = []
        for i in range(3):
            rt = cpool.tile([1, D + 2], BF16, name=f"r_t{i}")
            nc.sync.dma_start(rt[:], wp_in[5 * P + i : 5 * P + i + 1, :])
            r_t.append(rt)

        # edge meta, loaded and decoded once
        meta_sb = cpool.tile([P, sum_n], I32, name="meta_sb")
        nc.sync.dma_start(meta_sb[:], meta_in)
        srcs_all = cpool.tile([P, sum_n], I32, name="srcs_all")
        nc.vector.tensor_scalar(srcs_all[:], meta_sb[:], 0xFFFF, None, op0=OP.bitwise_and)
        # (>>16) & 0xFFFF: the mask recovers the unsigned field even if the
        # shift sign-extends (numpy and possibly the DVE do arithmetic shifts
        # on int32 regardless of the "logical" op name)
        grow_all = cpool.tile([P, sum_n], I32, name="grow_all")
        nc.vector.tensor_scalar(
            grow_all[:],
            meta_sb[:],
            16,
            0xFFFF,
            op0=OP.logical_shift_right,
            op1=OP.bitwise_and,
        )
        loc_all = cpool.tile([P, sum_n], I32, name="loc_all")
        nc.vector.tensor_scalar(loc_all[:], grow_all[:], 0x7F, None, op0=OP.bitwise_and)
        locf_all = cpool.tile([P, sum_n], F32, name="locf_all")
        nc.vector.tensor_copy(locf_all[:], loc_all[:])

        def phase_a(srcs, rrow):
            for s in range(tiles_per_core):
                r0 = s * P
                pa = pp.tile([P, D + 2], F32, tag="pa")
                nmm = len(srcs)
                for k, src in enumerate(srcs):
                    if src[0] == "q8":
                        _, q_ap, wt = src
                        xq = apool.tile([P, D], I8, tag="xq")
                        nc.sync.dma_start(xq[:], q_ap[r0 : r0 + P, 0:D])
                        xs = apool.tile([P, 1], F32, tag="xs")
                        nc.sync.dma_start(
                            xs[:], q_ap[r0 : r0 + P, D : D + 4].bitcast(F32)
                        )
                        xb = apool.tile([P, D], BF16, tag="xb")
                        nc.vector.tensor_scalar(
                            xb[:], xq[:], xs[:, 0:1], None, op0=OP.mult
                        )
                        pt = pp.tile([P, P], BF16, tag="pt")
                        nc.tensor.transpose(pt[:], xb[:], ident_t[:])
                        xt = apool.tile([P, P], BF16, tag="xt")
                        nc.vector.tensor_copy(xt[:], pt[:])
                    else:
                        _, src_ap, wt = src
                        xt = apool.tile([P, P], BF16, tag="xt")
                        nc.sync.dma_start(xt[:], src_ap[r0 : r0 + P, :], transpose=True)
                    nc.tensor.matmul(
                        pa[:],
                        lhsT=xt[:],
                        rhs=wt[:],
                        start=(k == 0),
                        stop=(k == nmm - 1 and rrow is None),
                    )
                if rrow is not None:
                    nc.tensor.matmul(
                        pa[:], lhsT=ones_row[:], rhs=rrow[:], start=False, stop=True
                    )
                ob = apool.tile([P, ROWB], BF16, tag="ob")
                nc.vector.tensor_copy(ob[:, 0:D], pa[:, 0:D])
                nc.vector.memset(ob[:, COL_ONES : COL_ONES + 1], 1.0)
                nc.vector.tensor_copy(ob[:, COL_SSRC : COL_SDST + 1], pa[:, D : D + 2])
                nc.vector.memset(ob[:, COL_SDST + 1 : ROWB], 0.0)
                nc.sync.dma_start(ag_in[r0 : r0 + P, :], ob[:])

        def phase_b(haug, y_dst, ys_dst=None, dbg_this=None):
            col0 = 0
            for s in range(tiles_per_core):
                n = n_chunks[s]
                # the HW indirect DMA honors ONE offset per partition (it
                # gathers out-row-size contiguous bytes from it), so issue one
                # DMA per 128-edge chunk into slices of the shared tiles
                g = gpool.tile([P, n * ROWB], BF16, tag="G")
                zt = epool.tile([P, n], BF16, tag="zt")
                for c in range(n):
                    nc.gpsimd.indirect_dma_start(
                        out=g[:, c * ROWB : (c + 1) * ROWB],
                        out_offset=None,
                        in_=haug[:],
                        in_offset=bass.IndirectOffsetOnAxis(
                            ap=srcs_all[:, col0 + c : col0 + c + 1], axis=0
                        ),
                    )
                    nc.gpsimd.indirect_dma_start(
                        out=zt[:, c : c + 1],
                        out_offset=None,
                        in_=haug[:],
                        in_offset=bass.IndirectOffsetOnAxis(
                            ap=grow_all[:, col0 + c : col0 + c + 1], axis=0
                        ),
                        element_offset=COL_SDST,
                    )
                gv = g[:].rearrange("p (n r) -> p n r", r=ROWB)
                z = epool.tile([P, n], F32, tag="z")
                nc.vector.tensor_tensor(
                    z[:], gv[:, :, COL_SSRC : COL_SSRC + 1].opt(), zt[:], op=OP.add
                )
                es = epool.tile([P, n], F32, tag="es")
                nc.vector.tensor_scalar(es[:], z[:], NEG_SLOPE, None, op0=OP.mult)
                el = epool.tile([P, n], F32, tag="el")
                nc.vector.tensor_tensor(el[:], es[:], z[:], op=OP.max)
                ex = epool.tile([P, n], F32, tag="ex")
                nc.scalar.activation(ex[:], el[:], ACTF.Exp)
                pacc = ppb.tile([P, D + 1], F32, tag="pacc")
                for c in range(n):
                    sw = swpool.tile([P, P], BF16, tag="sw")
                    nc.vector.tensor_scalar(
                        sw[:],
                        iota_f[:],
                        locf_all[:, col0 + c : col0 + c + 1],
                        ex[:, c : c + 1],
                        op0=OP.is_equal,
                        op1=OP.mult,
                    )
                    nc.tensor.matmul(
                        pacc[:],
                        lhsT=sw[:],
                        rhs=g[:, c * ROWB : c * ROWB + D + 1],
                        start=(c == 0),
                        stop=(c == n - 1),
                    )
                if dbg_this is not None and s == 0:
                    nc.sync.dma_start(dbg_this["g"], g[:])
                    nc.sync.dma_start(dbg_this["zt"], zt[:])
                    nc.sync.dma_start(dbg_this["ex"], ex[:])
                    nc.sync.dma_start(dbg_this["z"], z[:])
                    nc.sync.dma_start(dbg_this["srcs"], srcs_all[:, col0 : col0 + n])
                    nc.sync.dma_start(dbg_this["grow"], grow_all[:, col0 : col0 + n])
                den = epool.tile([P, 1], F32, tag="den")
                nc.vector.tensor_scalar(den[:], pacc[:, D : D + 1], 1e-30, None, op0=OP.add)
                rden = epool.tile([P, 1], F32, tag="rden")
                nc.vector.reciprocal(rden[:], den[:])
                if ys_dst is None:
                    ot = epool.tile([P, D], BF16, tag="ot")
                    nc.vector.tensor_scalar(
                        ot[:], pacc[:, 0:D], rden[:, 0:1], None, op0=OP.mult
                    )
                    nc.sync.dma_start(y_dst[s * P : (s + 1) * P, :], ot[:])
                else:
                    # final layer: emit int8 rows + per-row scale
                    otf = epool.tile([P, D], F32, tag="otf")
                    nc.vector.tensor_scalar(
                        otf[:], pacc[:, 0:D], rden[:, 0:1], None, op0=OP.mult
                    )
                    rmax = epool.tile([P, 1], F32, tag="rmax")
                    nc.vector.tensor_reduce(
                        rmax[:],
                        otf[:],
                        axis=mybir.AxisListType.X,
                        op=OP.max,
                        apply_absolute_value=True,
                    )
                    qs = epool.tile([P, 1], F32, tag="qs")
                    nc.vector.tensor_scalar(
                        qs[:], rmax[:], 1e-20, 1.0 / 127.0, op0=OP.max, op1=OP.mult
                    )
                    rqs = epool.tile([P, 1], F32, tag="rqs")
                    nc.vector.reciprocal(rqs[:], qs[:])
                    yq = epool.tile([P, D], I8, tag="yq")
                    nc.vector.tensor_scalar(
                        yq[:], otf[:], rqs[:, 0:1], None, op0=OP.mult
                    )
                    nc.sync.dma_start(y_dst[s * P : (s + 1) * P, 0:D], yq[:])
                    nc.sync.dma_start(
                        y_dst[s * P : (s + 1) * P, D : D + 4], qs[:].bitcast(I8)
                    )
                col0 += n

        layers = [
            ([("q8", xp_in, w_t[0])], None, haugs[0], y_mid[0], None),
            ([("bf", y_mid[0][:], w_t[1])], r_t[0], haugs[1], y_mid[1], None),
            (
                [("bf", y_mid[1][:], w_t[2]), ("bf", y_mid[0][:], w_t[3])],
                r_t[1],
                haugs[2],
                y_mid[2],
                None,
            ),
            ([("bf", y_mid[2][:], w_t[4])], r_t[2], haugs[3], y_out, True),
        ]
        for li, (srcs, rrow, hb, ydst, ysdst) in enumerate(layers):
            phase_a(srcs, rrow)
            nc.gpsimd.collective_compute(
                "AllGather",
                OP.bypass,
                replica_groups=[list(range(n_cores))],
                ins=[ag_in[:].opt()],
                outs=[hb[:].opt()],
            )
            phase_b(
                hb[:],
                ydst,
                ys_dst=ysdst,
                dbg_this=(dbg if debug_dump and li == 0 else None),
            )
            if debug_dump and li == 0:
                nc.sync.dma_start(dbg_haug, hb[:])
                nc.sync.dma_start(dbg_u0, y_mid[0][:])

    nc.compile()
    return nc


def _make_runner(nc, n_cores):
    install_neuronx_cc_hook()
    partition_name = nc.partition_id_tensor.name if nc.partition_id_tensor else None
    in_names, out_names, out_avals, zero_shapes = [], [], [], []
    for alloc in nc.m.functions[0].allocations:
        if not isinstance(alloc, mybir.MemoryLocationSet):
            continue
        name = alloc.memorylocations[0].name
        if alloc.kind == "ExternalInput":
            if name != partition_name:
                in_names.append(name)
        elif alloc.kind == "ExternalOutput":
            shape = tuple(alloc.tensor_shape)
            dtype = mybir.dt.np(alloc.dtype)
            out_names.append(name)
            out_avals.append(jax.core.ShapedArray(shape, dtype))
            zero_shapes.append((shape, dtype))
    n_params = len(in_names)
    all_in_names = list(in_names) + list(out_names)
    if partition_name is not None:
        all_in_names.append(partition_name)

    def _body(*args):
        operands = list(args)
        if partition_name is not None:
            operands.append(partition_id_tensor())
        outs = _bass_exec_p.bind(
            *operands,
            out_avals=tuple(out_avals),
            in_names=tuple(all_in_names),
            out_names=tuple(out_names),
            lowering_input_output_aliases=(),
            sim_require_finite=False,
            sim_require_nnan=False,
            nc=nc,
        )
        return tuple(outs)

    devices = jax.devices()[:n_cores]
    mesh = Mesh(np.asarray(devices), ("core",))
    n_outs = len(out_names)
    in_specs = (PartitionSpec("core"),) * (n_params + n_outs)
    out_specs = (PartitionSpec("core"),) * n_outs
    sharded = jax.jit(
        shard_map(
            _body, mesh=mesh, in_specs=in_specs, out_specs=out_specs, check_rep=False
        ),
        keep_unused=True,
    )
    # persistent (non-donated) output operand buffers, staged once
    concat_zeros = [
        jax.device_put(
            np.zeros((n_cores * shp[0], *shp[1:]), dt),
            NamedSharding(mesh, PartitionSpec("core")),
        )
        for shp, dt in zero_shapes
    ]
    jax.block_until_ready(concat_zeros)
    return sharded, in_names, out_names, concat_zeros


_CACHE = {}


def _get_ctx(edge_index, n_nodes, n_cores):
    key = (n_nodes, n_cores, hash(np.asarray(edge_index).tobytes()))
    if _CACHE.get("key") != key:
        tiles_per_core, n_pad, n_chunks, metas = _prep_graph(
            edge_index, n_nodes, n_cores
        )
        nc = _build_program(tiles_per_core, n_chunks, n_cores)
        sharded, in_names, out_names, dev_zeros = _make_runner(nc, n_cores)
        # meta is a pure function of edge_index (the cache key), so it can be
        # staged on-device once alongside the compiled program
        devices = jax.devices()[:n_cores]
        mesh = Mesh(np.asarray(devices), ("core",))
        sharding = NamedSharding(mesh, PartitionSpec("core"))
        dev_meta = jax.device_put(np.concatenate(metas, axis=0), sharding)
        jax.block_until_ready(dev_meta)
        _CACHE.clear()
        _CACHE["key"] = key
        _CACHE["ctx"] = (
            tiles_per_core,
            n_pad,
            dev_meta,
            sharded,
            in_names,
            out_names,
            dev_zeros,
            devices,
            sharding,
        )
    return _CACHE["ctx"]


def kernel(**inputs):
    edge_index = np.asarray(inputs["edge_index"])
    (
        tiles_per_core,
        n_pad,
        dev_meta,
        sharded,
        in_names,
        out_names,
        dev_zeros,
        devices,
        sharding,
    ) = _get_ctx(edge_index, N_FULL, N_CORES)
    npc = tiles_per_core * P

    x = np.asarray(inputs["x"], np.float32)
    xp = np.zeros((n_pad, D + 4), dtype=np.int8)

    def _quant_core(i):
        # quantize core i's rows, then ship its shard immediately
        # (pad rows keep q=0 and scale=0 -> dequantize to 0)
        a, b = i * npc, min((i + 1) * npc, N_FULL)
        if a < b:
            xs_blk = np.abs(x[a:b]).max(axis=1)
            xs_blk[xs_blk == 0] = 1.0
            xs_blk = (xs_blk / 127.0).astype(np.float32)
            xp[a:b, 0:D] = np.round(x[a:b] * (1.0 / xs_blk)[:, None]).astype(np.int8)
            xp[a:b, D : D + 4] = xs_blk.view(np.int8).reshape(-1, 4)
        return jax.device_put(xp[i * npc : (i + 1) * npc], devices[i])

    Wp = np.asarray(inputs["Wp"], np.float32)
    Wp1, Wp2 = Wp[:D], Wp[D:]
    aug_e = _aug(inputs["We"], inputs["ae_s"], inputs["ae_d"])
    aug_p1s = _aug(Wp1 + Wp2, inputs["ap_s"], inputs["ap_d"])
    aug_p1 = _aug(Wp1, inputs["ap_s"], inputs["ap_d"])
    aug_p2 = _aug(Wp2, inputs["ap_s"], inputs["ap_d"])
    aug_d = _aug(inputs["Wd"], inputs["ad_s"], inputs["ad_d"])
    be = np.asarray(inputs["be"], np.float32)
    bp = np.asarray(inputs["bp"], np.float32)
    bd = np.asarray(inputs["bd"], np.float32)
    r0 = (be @ aug_p1s)[None, :]
    r1 = (bp @ aug_p1 + be @ aug_p2)[None, :]
    r2 = (bp @ aug_d)[None, :]

    wpack = np.ascontiguousarray(
        np.concatenate(
            [aug_e, aug_p1s, aug_p1, aug_p2, aug_d, r0, r1, r2], axis=0
        ).astype(NPBF16)
    )
    # quantize+upload x_p's 8 shards and the (replicated) weight pack
    # concurrently — per-transfer fixed cost dominates on the tunnel
    with _cf.ThreadPoolExecutor(2 * N_CORES) as ex:
        xfut = [ex.submit(_quant_core, i) for i in range(N_CORES)]
        wfut = [ex.submit(jax.device_put, wpack, devices[i]) for i in range(N_CORES)]
        xparts = [f.result() for f in xfut]
        wparts = [f.result() for f in wfut]
    x_dev = jax.make_array_from_single_device_arrays(xp.shape, sharding, xparts)
    w_dev = jax.make_array_from_single_device_arrays(
        (N_CORES * wpack.shape[0], wpack.shape[1]), sharding, wparts
    )
    vals = {"x_p": x_dev, "meta": dev_meta, "w_p": w_dev}
    concat_in = [vals[name] for name in in_names]

    out_arrs = sharded(*concat_in, *dev_zeros)
    yp_arr = out_arrs[out_names.index("y_p")]
    # fetch the 8 shards concurrently and dequantize each as it lands
    shards = sorted(yp_arr.addressable_shards, key=lambda s: s.index[0].start or 0)
    y = np.empty((len(shards) * npc, D), dtype=np.float32)

    def _fetch_deq(i):
        yp = np.asarray(shards[i].data)
        ys = np.ascontiguousarray(yp[:, D : D + 4]).view(np.float32)
        y[i * npc : (i + 1) * npc] = yp[:, 0:D].astype(np.float32) * ys

    with _cf.ThreadPoolExecutor(len(shards)) as ex:
        list(ex.map(_fetch_deq, range(len(shards))))
    yout = y[:N_FULL]
    yout += bd[None, :]
    return yout
